# revision 1
# baseline (speedup 1.0000x reference)
"""ForwardDeformer Broyden-search kernel for TRN2 (8 cores, data-parallel over points).

Per-core layout:
  - N_pad = 128*PPP points; point-inits along free dim as (t, i), i (13 bones) inner.
  - Tiles chunk t into Tp pieces: tile free width F = Tp*13 point-inits/partition.
  - State (xc, gx, Ji, conv, div) lives in SBUF pools (bufs=2 -> 2 tiles in flight).
  - Trilinear gathers: indirect-DMA descriptors from DRAM AoS tables
    vox_d [cell*3+c] (6-f32 descriptors: x-pair) and vox_J [cell*9+k]
    (18-f32 descriptors), 4 descriptors (z,y corners) per point-interp.
  - Float semantics mirror the jax reference op-for-op where possible.
"""
import sys
sys.path.insert(0, '/opt/trn_rl_repo')
import numpy as np
import concourse.bass as bass
import concourse.tile as tile
from concourse import bacc, mybir
from concourse.bass import AP
f32 = mybir.dt.float32
i32 = mybir.dt.int32
u8 = mybir.dt.uint8
Alu = mybir.AluOpType

D, H, W = 32, 128, 128
NCELL = D * H * W
NJ = 24
NI = 13
RATIO = 4.0
CVG2 = float(np.float32(1e-5)) ** 2
DVG2 = float(np.float32(0.1)) ** 2
DUP_EPS2 = 1e-8


def mk_ap(t, offset, dims):
    h = t.tensor if isinstance(t, AP) else t
    return AP(tensor=h, offset=offset, ap=[[int(s), int(c)] for s, c in dims])


def pstr(t):
    return t[:].ap[0][0]


def build_nc(PPP=196, Tp=28, NITER=10, STATE_BUFS=2, GQ=4, JSUB=7, PREAGG=4):
    assert PPP % Tp == 0
    NT = PPP // Tp
    F = Tp * NI
    assert F % GQ == 0 and F % JSUB == 0
    FH = F // GQ             # d-gather chunk width (point-inits)
    assert FH % NI == 0
    FS = F // JSUB           # J-gather chunk width

    nc = bacc.Bacc("TRN2", target_bir_lowering=False, num_devices=8)

    xd_in = nc.declare_dram_parameter("xd_in", [128, PPP * 3], f32, isOutput=False)
    lbs = nc.declare_dram_parameter("lbs", [NJ, NCELL], f32, isOutput=False)
    a_in = nc.declare_dram_parameter("a_in", [NJ, 12], f32, isOutput=False)
    tinv_in = nc.declare_dram_parameter("tinv_in", [1, NI * 12], f32, isOutput=False)
    off_in = nc.declare_dram_parameter("off_in", [1, 3], f32, isOutput=False)
    scale_in = nc.declare_dram_parameter("scale_in", [1, 1], f32, isOutput=False)

    xc_out = nc.declare_dram_parameter("xc_out", [128, PPP * NI * 3], f32, isOutput=True)
    ji_out = nc.declare_dram_parameter("ji_out", [128, PPP * NI * 9], f32, isOutput=True)
    va_out = nc.declare_dram_parameter("va_out", [128, PPP * NI], u8, isOutput=True)

    vox_d = nc.dram_tensor("vox_d", [NCELL * 3, 1], f32)
    vox_j = nc.dram_tensor("vox_j", [NCELL * 9, 1], f32)

    GRPS = 42                       # groups per PSUM bank (504 cols)
    NGRP = NCELL // 128             # 4096
    NSUPER = (NGRP + GRPS * PREAGG - 1) // (GRPS * PREAGG)

    with tile.TileContext(nc) as tc:
        with tc.tile_pool(name="consts", bufs=1) as cpool:
            V = nc.vector
            G = nc.gpsimd

            def floor_into(dst, src, itmp, ftmp):
                """dst = floor(src) (src >= 0), conversion-mode independent."""
                V.tensor_copy(itmp, src)
                V.tensor_copy(dst, itmp)
                V.tensor_tensor(ftmp, dst, src, op=Alu.is_gt)
                V.tensor_tensor(dst, dst, ftmp, op=Alu.subtract)

            # ================= constants =================
            offb = cpool.tile([128, 3], f32, tag="offb", name="offb")
            nc.sync.dma_start(offb[:], off_in[:].to_broadcast([128, 3]))
            scb = cpool.tile([128, 1], f32, tag="scb", name="scb")
            nc.sync.dma_start(scb[:], scale_in[:].to_broadcast([128, 1]))
            rs = cpool.tile([128, 4], f32, tag="rs", name="rs")
            V.reciprocal(rs[:, 0:1], scb[:])
            V.tensor_copy(rs[:, 1:2], rs[:, 0:1])
            V.tensor_copy(rs[:, 2:3], rs[:, 0:1])
            V.tensor_single_scalar(rs[:, 3:4], rs[:, 0:1], RATIO, op=Alu.mult)
            tinvb = cpool.tile([128, NI * 12], f32, tag="tinvb", name="tinvb")
            nc.sync.dma_start(tinvb[:], tinv_in[:].to_broadcast([128, NI * 12]))
            scpre = cpool.tile([128, 3], f32, tag="scpre", name="scpre")
            V.tensor_copy(scpre[:, 0:1], scb[:])
            V.tensor_copy(scpre[:, 1:2], scb[:])
            V.tensor_single_scalar(scpre[:, 2:3], scb[:], 1.0 / RATIO, op=Alu.mult)
            a_t = cpool.tile([NJ, 12], f32, tag="a_t", name="a_t")
            nc.sync.dma_start(a_t[:], a_in[:])
            pxi = cpool.tile([128, 1], i32, tag="pxi", name="pxi")
            G.iota(pxi[:], pattern=[[1, 1]], base=0, channel_multiplier=1)
            pxd = cpool.tile([128, 1], f32, tag="pxd", name="pxd")
            V.tensor_copy(pxd[:], pxi[:])
            V.tensor_scalar(pxd[:], pxd[:], 2.0 / 127.0, -1.0, op0=Alu.mult, op1=Alu.add)
            V.tensor_tensor(pxd[:], pxd[:], scpre[:, 0:1], op=Alu.mult)
            V.tensor_tensor(pxd[:], pxd[:], offb[:, 0:1], op=Alu.add)

            def rs_bc(c, width):
                return mk_ap(rs[:], 1 + c, [[pstr(rs), 128], [0, width]])

            # ================= precompute =================
            with (
                tc.tile_pool(name="pre", bufs=2) as ppool,
                tc.tile_pool(name="preb", bufs=2) as pbpool,
                tc.tile_pool(name="prepsum", bufs=2, space="PSUM") as pspool,
            ):
                for sup in range(NSUPER):
                    sup_grp = sup * GRPS * PREAGG
                    gtot = min(GRPS * PREAGG, NGRP - sup_grp)
                    tb = pbpool.tile([128, GRPS * PREAGG * 12], f32, tag="tb", name="tb")
                    for sub in range((gtot + GRPS - 1) // GRPS):
                        base_grp = sup_grp + sub * GRPS
                        gh = min(GRPS, NGRP - base_grp)
                        lchunk = ppool.tile([NJ, 128 * GRPS], f32, tag="lchunk", name="lchunk")
                        nc.sync.dma_start(
                            lchunk[:, :128 * gh],
                            mk_ap(lbs, base_grp * 128, [[NCELL, NJ], [1, 128 * gh]]))
                        tpsum = pspool.tile([128, GRPS * 12], f32, tag="tpsum",
                                            name="tpsum", space="PSUM")
                        for g in range(gh):
                            nc.tensor.matmul(
                                tpsum[:, g * 12:(g + 1) * 12],
                                lhsT=lchunk[:, g * 128:(g + 1) * 128],
                                rhs=a_t[:], start=True, stop=True)
                        V.tensor_copy(tb[:, sub * GRPS * 12:sub * GRPS * 12 + gh * 12],
                                      tpsum[:, :gh * 12])

                    def T(k):
                        return mk_ap(tb[:], k, [[pstr(tb), 128], [12, gtot]])
                    ggi = ppool.tile([128, GRPS * PREAGG], i32, tag="ggi", name="ggi")
                    G.iota(ggi[:, :gtot], pattern=[[1, gtot]], base=sup_grp,
                           channel_multiplier=0)
                    ggf = ppool.tile([128, GRPS * PREAGG], f32, tag="ggf", name="ggf")
                    V.tensor_copy(ggf[:, :gtot], ggi[:, :gtot])
                    zf = ppool.tile([128, GRPS * PREAGG], f32, tag="zf", name="zf")
                    yf = ppool.tile([128, GRPS * PREAGG], f32, tag="yf", name="yf")
                    zit = ppool.tile([128, GRPS * PREAGG], i32, tag="zit", name="zit")
                    tmp1 = ppool.tile([128, GRPS * PREAGG], f32, tag="tmp1", name="tmp1")
                    tmp2 = ppool.tile([128, GRPS * PREAGG], f32, tag="tmp2", name="tmp2")
                    t1, t2 = tmp1[:, :gtot], tmp2[:, :gtot]
                    V.tensor_single_scalar(t1, ggf[:, :gtot], 1.0 / 128.0, op=Alu.mult)
                    floor_into(zf[:, :gtot], t1, zit[:, :gtot], t2)
                    V.tensor_scalar(yf[:, :gtot], zf[:, :gtot], -128.0, 0.0,
                                    op0=Alu.mult, op1=Alu.add)
                    V.tensor_tensor(yf[:, :gtot], yf[:, :gtot], ggf[:, :gtot], op=Alu.add)
                    pyd = ppool.tile([128, GRPS * PREAGG], f32, tag="pyd", name="pyd")
                    V.tensor_scalar(pyd[:, :gtot], yf[:, :gtot], 2.0 / 127.0, -1.0,
                                    op0=Alu.mult, op1=Alu.add)
                    V.tensor_tensor(pyd[:, :gtot], pyd[:, :gtot],
                                    mk_ap(scpre[:], 1, [[pstr(scpre), 128], [0, gtot]]), op=Alu.mult)
                    V.tensor_tensor(pyd[:, :gtot], pyd[:, :gtot],
                                    mk_ap(offb[:], 1, [[pstr(offb), 128], [0, gtot]]), op=Alu.add)
                    pzd = ppool.tile([128, GRPS * PREAGG], f32, tag="pzd", name="pzd")
                    V.tensor_scalar(pzd[:, :gtot], zf[:, :gtot], 2.0 / 31.0, -1.0,
                                    op0=Alu.mult, op1=Alu.add)
                    V.tensor_tensor(pzd[:, :gtot], pzd[:, :gtot],
                                    mk_ap(scpre[:], 2, [[pstr(scpre), 128], [0, gtot]]), op=Alu.mult)
                    V.tensor_tensor(pzd[:, :gtot], pzd[:, :gtot],
                                    mk_ap(offb[:], 2, [[pstr(offb), 128], [0, gtot]]), op=Alu.add)
                    pxdb = mk_ap(pxd[:], 0, [[pstr(pxd), 128], [0, gtot]])

                    vb = pbpool.tile([128, GRPS * PREAGG * 12], f32, tag="vb", name="vb")

                    def O(k):
                        return mk_ap(vb[:], k, [[pstr(vb), 128], [12, gtot]])
                    for c in range(3):
                        V.tensor_tensor(t1, T(c * 4 + 0), pxdb, op=Alu.mult)
                        V.tensor_tensor(t2, T(c * 4 + 1), pyd[:, :gtot], op=Alu.mult)
                        V.tensor_tensor(t1, t1, t2, op=Alu.add)
                        V.tensor_tensor(t2, T(c * 4 + 2), pzd[:, :gtot], op=Alu.mult)
                        V.tensor_tensor(t1, t1, t2, op=Alu.add)
                        V.tensor_tensor(O(c), t1, T(c * 4 + 3), op=Alu.add)
                    M = lambda r, c_: T(r * 4 + c_)
                    det = ppool.tile([128, GRPS * PREAGG], f32, tag="det", name="det")
                    dt_ = det[:, :gtot]
                    for c0 in range(3):
                        for r0 in range(3):
                            rr = [x for x in range(3) if x != r0]
                            cc = [x for x in range(3) if x != c0]
                            V.tensor_tensor(t1, M(rr[0], cc[0]), M(rr[1], cc[1]), op=Alu.mult)
                            V.tensor_tensor(t2, M(rr[0], cc[1]), M(rr[1], cc[0]), op=Alu.mult)
                            if (r0 + c0) % 2 == 0:
                                V.tensor_tensor(O(3 + c0 * 3 + r0), t1, t2, op=Alu.subtract)
                            else:
                                V.tensor_tensor(O(3 + c0 * 3 + r0), t2, t1, op=Alu.subtract)
                    V.tensor_tensor(dt_, M(0, 0), O(3 + 0), op=Alu.mult)
                    V.tensor_tensor(t1, M(0, 1), O(3 + 3), op=Alu.mult)
                    V.tensor_tensor(dt_, dt_, t1, op=Alu.add)
                    V.tensor_tensor(t1, M(0, 2), O(3 + 6), op=Alu.mult)
                    V.tensor_tensor(dt_, dt_, t1, op=Alu.add)
                    V.reciprocal(dt_, dt_)
                    for k in range(9):
                        V.tensor_tensor(O(3 + k), O(3 + k), dt_, op=Alu.mult)
                    nc.sync.dma_start(
                        mk_ap(vox_d, sup_grp * 128 * 3, [[3, 128], [128 * 3, gtot], [1, 3]]),
                        mk_ap(vb[:], 0, [[pstr(vb), 128], [12, gtot], [1, 3]]))
                    nc.sync.dma_start(
                        mk_ap(vox_j, sup_grp * 128 * 9, [[9, 128], [128 * 9, gtot], [1, 9]]),
                        mk_ap(vb[:], 3, [[pstr(vb), 128], [12, gtot], [1, 9]]))

            # ================= Broyden =================
            with (
                tc.tile_pool(name="state", bufs=STATE_BUFS) as spool,
                tc.tile_pool(name="work", bufs=2) as wpool,
                tc.tile_pool(name="gath", bufs=2) as gpool,
                tc.tile_pool(name="gathj", bufs=1) as gjpool,
            ):
                for tl in range(NT):
                    t0 = tl * Tp
                    xc = [spool.tile([128, F], f32, tag=f"xc{c}", name=f"xc{c}") for c in range(3)]
                    gx = [spool.tile([128, F], f32, tag=f"gx{c}", name=f"gx{c}") for c in range(3)]
                    Ji = [spool.tile([128, F], f32, tag=f"ji{k}", name=f"ji{k}") for k in range(9)]
                    cv = spool.tile([128, F], f32, tag="cv", name="cv")
                    dv = spool.tile([128, F], f32, tag="dv", name="dv")
                    xd = spool.tile([128, Tp * 3], f32, tag="xd", name="xd")
                    nc.sync.dma_start(
                        xd[:], mk_ap(xd_in, t0 * 3, [[PPP * 3, 128], [1, Tp * 3]]))

                    def xd_c(c, lo=0, n=Tp):
                        return mk_ap(xd[:], lo * 3 + c, [[pstr(xd), 128], [3, n], [0, NI]])

                    def tinv_c(k):
                        return mk_ap(tinvb[:], k, [[pstr(tinvb), 128], [0, Tp], [12, NI]])

                    def as3(t_, off=0, n=Tp):
                        return mk_ap(t_[:], off * NI, [[pstr(t_), 128], [NI, n], [1, NI]])

                    tmpa = wpool.tile([128, F], f32, tag="tmpa", name="tmpa")
                    tmpb = wpool.tile([128, F], f32, tag="tmpb", name="tmpb")

                    # ---- xc0 ----
                    for c in range(3):
                        V.tensor_tensor(as3(tmpa), xd_c(0), tinv_c(c * 4 + 0), op=Alu.mult)
                        V.tensor_tensor(as3(tmpb), xd_c(1), tinv_c(c * 4 + 1), op=Alu.mult)
                        V.tensor_tensor(tmpa[:], tmpa[:], tmpb[:], op=Alu.add)
                        V.tensor_tensor(as3(tmpb), xd_c(2), tinv_c(c * 4 + 2), op=Alu.mult)
                        V.tensor_tensor(tmpa[:], tmpa[:], tmpb[:], op=Alu.add)
                        V.tensor_tensor(as3(xc[c]), as3(tmpa), tinv_c(c * 4 + 3), op=Alu.add)

                    # ---- per-tile temps ----
                    fx = wpool.tile([128, F], f32, tag="fx", name="fx")
                    x0 = wpool.tile([128, F], f32, tag="x0", name="x0")
                    cvt_i = wpool.tile([128, F], i32, tag="cvt_i", name="cvt_i")
                    gtt = wpool.tile([128, F], f32, tag="gtt", name="gtt")
                    ys0 = wpool.tile([128, F], f32, tag="ys0", name="ys0")
                    ys1 = wpool.tile([128, F], f32, tag="ys1", name="ys1")
                    zs0 = wpool.tile([128, F], f32, tag="zs0", name="zs0")
                    zs1 = wpool.tile([128, F], f32, tag="zs1", name="zs1")
                    xs = wpool.tile([128, F], f32, tag="xs", name="xs")
                    tx = wpool.tile([128, F], f32, tag="tx", name="tx")
                    ty = wpool.tile([128, F], f32, tag="ty", name="ty")
                    tz = wpool.tile([128, F], f32, tag="tz", name="tz")
                    idxi = wpool.tile([128, 4 * F], i32, tag="idxi", name="idxi")
                    w8 = [[wpool.tile([128, F], f32, tag=f"w8_{k}_{xp}", name=f"w8_{k}_{xp}")
                           for xp in range(2)] for k in range(4)]
                    cmb0 = wpool.tile([128, F], f32, tag="cmb0", name="cmb0")
                    accv = wpool.tile([128, F], f32, tag="accv", name="accv")
                    gnn = wpool.tile([128, F], f32, tag="gnn", name="gnn")
                    act = wpool.tile([128, F], f32, tag="act", name="act")
                    dgx = [wpool.tile([128, F], f32, tag=f"dgx{c}", name=f"dgx{c}") for c in range(3)]
                    dxt = [wpool.tile([128, F], f32, tag=f"dxt{c}", name=f"dxt{c}") for c in range(3)]
                    jdg = [wpool.tile([128, F], f32, tag=f"jdg{c}", name=f"jdg{c}") for c in range(3)]

                    def idx_weights():
                        for c, (K, mx, frac) in enumerate(
                                [(63.5, 126.0, tx), (63.5, 127.0, ty), (15.5, 31.0, tz)]):
                            nc.vector.scalar_tensor_tensor(
                                fx[:], xc[c][:], offb[:, c:c + 1], rs_bc(c, F),
                                op0=Alu.subtract, op1=Alu.mult)
                            V.tensor_scalar(fx[:], fx[:], -1.0, 1.0, op0=Alu.max, op1=Alu.min)
                            V.tensor_scalar(fx[:], fx[:], 1.0, K, op0=Alu.add, op1=Alu.mult)
                            floor_into(x0[:], fx[:], cvt_i[:], gtt[:])
                            V.tensor_tensor(frac[:], fx[:], x0[:], op=Alu.subtract)
                            if c == 0:
                                V.tensor_single_scalar(gtt[:], x0[:], mx, op=Alu.min)
                                V.tensor_tensor(x0[:], x0[:], gtt[:], op=Alu.subtract)
                                V.tensor_tensor(frac[:], frac[:], x0[:], op=Alu.add)
                                V.tensor_single_scalar(xs[:], gtt[:], 3.0, op=Alu.mult)
                            elif c == 1:
                                V.tensor_single_scalar(ys0[:], x0[:], 384.0, op=Alu.mult)
                                V.tensor_scalar(ys1[:], x0[:], 1.0, mx, op0=Alu.add, op1=Alu.min)
                                V.tensor_single_scalar(ys1[:], ys1[:], 384.0, op=Alu.mult)
                            else:
                                V.tensor_single_scalar(zs0[:], x0[:], 49152.0, op=Alu.mult)
                                V.tensor_scalar(zs1[:], x0[:], 1.0, mx, op0=Alu.add, op1=Alu.min)
                                V.tensor_single_scalar(zs1[:], zs1[:], 49152.0, op=Alu.mult)
                        for k, (zz, yy) in enumerate([(zs0, ys0), (zs0, ys1), (zs1, ys0), (zs1, ys1)]):
                            V.tensor_tensor(cmb0[:], zz[:], yy[:], op=Alu.add)
                            V.tensor_tensor(cmb0[:], cmb0[:], xs[:], op=Alu.add)
                            V.tensor_copy(
                                mk_ap(idxi[:], k, [[pstr(idxi), 128], [4, F]]), cmb0[:])

                    def mk_w8():
                        # reuses gtt (1-ty), x0 ((1-tz) then (1-tz)ty), fx, ys0, ys1 as scratch
                        V.tensor_scalar(gtt[:], ty[:], -1.0, 1.0, op0=Alu.mult, op1=Alu.add)
                        V.tensor_scalar(x0[:], tz[:], -1.0, 1.0, op0=Alu.mult, op1=Alu.add)
                        V.tensor_tensor(fx[:], x0[:], gtt[:], op=Alu.mult)    # (1-tz)(1-ty)
                        V.tensor_tensor(x0[:], x0[:], ty[:], op=Alu.mult)     # (1-tz)ty
                        V.tensor_tensor(gtt[:], tz[:], gtt[:], op=Alu.mult)   # tz(1-ty)
                        V.tensor_tensor(ys0[:], tz[:], ty[:], op=Alu.mult)    # tz*ty
                        V.tensor_scalar(ys1[:], tx[:], -1.0, 1.0, op0=Alu.mult, op1=Alu.add)
                        for k, wzy_t in enumerate((fx, x0, gtt, ys0)):
                            V.tensor_tensor(w8[k][0][:], wzy_t[:], ys1[:], op=Alu.mult)
                            V.tensor_tensor(w8[k][1][:], wzy_t[:], tx[:], op=Alu.mult)

                    def gather_d():
                        out = []
                        for h in range(GQ):
                            gd = gpool.tile([128, FH * 24], f32, tag="gd", name="gd")
                            G.indirect_dma_start(
                                out=gd[:], out_offset=None, in_=vox_d[:],
                                in_offset=bass.IndirectOffsetOnAxis(
                                    ap=idxi[:, h * 4 * FH:(h + 1) * 4 * FH], axis=0))
                            out.append((gd, h * FH, FH))
                        return out

                    def combine_d(gd, lo, n, c, out_ap):
                        ps = pstr(gd)
                        sl = slice(lo, lo + n)
                        first = True
                        for k in range(4):
                            for xp in range(2):
                                g = mk_ap(gd[:], k * 6 + xp * 3 + c, [[ps, 128], [24, n]])
                                if first:
                                    V.tensor_tensor(out_ap, g, w8[k][xp][:, sl], op=Alu.mult)
                                    first = False
                                else:
                                    V.tensor_tensor(cmb0[:, sl], g, w8[k][xp][:, sl], op=Alu.mult)
                                    V.tensor_tensor(out_ap, out_ap, cmb0[:, sl], op=Alu.add)

                    # ---- init gathers ----
                    idx_weights()
                    mk_w8()
                    for gd, lo, n in gather_d():
                        sl = slice(lo, lo + n)
                        plo, pn = lo // NI, n // NI
                        for c in range(3):
                            combine_d(gd, lo, n, c, accv[:, sl])
                            V.tensor_tensor(as3(gx[c], plo, pn), as3(accv, plo, pn),
                                            xd_c(c, plo, pn), op=Alu.subtract)
                    # J gather: idxj = idx_d * 3
                    V.tensor_single_scalar(idxi[:], idxi[:], 3, op=Alu.mult)
                    for sb in range(JSUB):
                        gj = gjpool.tile([128, FS * 72], f32, tag="gj", name="gj")
                        G.indirect_dma_start(
                            out=gj[:], out_offset=None, in_=vox_j[:],
                            in_offset=bass.IndirectOffsetOnAxis(
                                ap=idxi[:, sb * 4 * FS:(sb + 1) * 4 * FS], axis=0))
                        ps = pstr(gj)
                        sl = slice(sb * FS, (sb + 1) * FS)
                        for k9 in range(9):
                            first = True
                            for k in range(4):
                                for xp in range(2):
                                    g = mk_ap(gj[:], k * 18 + xp * 9 + k9, [[ps, 128], [72, FS]])
                                    if first:
                                        V.tensor_tensor(Ji[k9][:, sl], g, w8[k][xp][:, sl], op=Alu.mult)
                                        first = False
                                    else:
                                        V.tensor_tensor(cmb0[:, sl], g, w8[k][xp][:, sl], op=Alu.mult)
                                        V.tensor_tensor(Ji[k9][:, sl], Ji[k9][:, sl], cmb0[:, sl], op=Alu.add)

                    # ---- init flags ----
                    V.tensor_tensor(gnn[:], gx[0][:], gx[0][:], op=Alu.mult)
                    V.tensor_tensor(tmpa[:], gx[1][:], gx[1][:], op=Alu.mult)
                    V.tensor_tensor(gnn[:], gnn[:], tmpa[:], op=Alu.add)
                    V.tensor_tensor(tmpa[:], gx[2][:], gx[2][:], op=Alu.mult)
                    V.tensor_tensor(gnn[:], gnn[:], tmpa[:], op=Alu.add)
                    V.tensor_single_scalar(cv[:], gnn[:], CVG2, op=Alu.is_lt)
                    V.memset(dv[:], 0.0)

                    # ---- iterations ----
                    for it in range(NITER):
                        V.tensor_tensor(act[:], cv[:], dv[:], op=Alu.max)
                        V.tensor_scalar(act[:], act[:], -1.0, 1.0, op0=Alu.mult, op1=Alu.add)
                        for c in range(3):
                            V.tensor_tensor(tmpa[:], Ji[c * 3 + 0][:], gx[0][:], op=Alu.mult)
                            V.tensor_tensor(tmpb[:], Ji[c * 3 + 1][:], gx[1][:], op=Alu.mult)
                            V.tensor_tensor(tmpa[:], tmpa[:], tmpb[:], op=Alu.add)
                            V.tensor_tensor(tmpb[:], Ji[c * 3 + 2][:], gx[2][:], op=Alu.mult)
                            V.tensor_tensor(tmpa[:], tmpa[:], tmpb[:], op=Alu.add)
                            V.tensor_single_scalar(tmpa[:], tmpa[:], -1.0, op=Alu.mult)
                            V.tensor_tensor(dxt[c][:], tmpa[:], act[:], op=Alu.mult)
                            V.tensor_tensor(xc[c][:], xc[c][:], dxt[c][:], op=Alu.add)
                        idx_weights()
                        mk_w8()
                        for gd, lo, n in gather_d():
                            sl = slice(lo, lo + n)
                            plo, pn = lo // NI, n // NI
                            for c in range(3):
                                combine_d(gd, lo, n, c, accv[:, sl])
                                V.tensor_tensor(as3(accv, plo, pn), as3(accv, plo, pn),
                                                xd_c(c, plo, pn), op=Alu.subtract)
                                V.tensor_tensor(dgx[c][:, sl], accv[:, sl], gx[c][:, sl],
                                                op=Alu.subtract)
                        for c in range(3):
                            V.tensor_tensor(tmpa[:], Ji[c * 3 + 0][:], dgx[0][:], op=Alu.mult)
                            V.tensor_tensor(tmpb[:], Ji[c * 3 + 1][:], dgx[1][:], op=Alu.mult)
                            V.tensor_tensor(tmpa[:], tmpa[:], tmpb[:], op=Alu.add)
                            V.tensor_tensor(tmpb[:], Ji[c * 3 + 2][:], dgx[2][:], op=Alu.mult)
                            V.tensor_tensor(jdg[c][:], tmpa[:], tmpb[:], op=Alu.add)
                        den = gnn
                        V.tensor_tensor(den[:], dxt[0][:], jdg[0][:], op=Alu.mult)
                        V.tensor_tensor(tmpa[:], dxt[1][:], jdg[1][:], op=Alu.mult)
                        V.tensor_tensor(den[:], den[:], tmpa[:], op=Alu.add)
                        V.tensor_tensor(tmpa[:], dxt[2][:], jdg[2][:], op=Alu.mult)
                        V.tensor_tensor(den[:], den[:], tmpa[:], op=Alu.add)
                        V.tensor_single_scalar(tmpa[:], den[:], 0.0, op=Alu.abs_max)
                        V.tensor_single_scalar(tmpa[:], tmpa[:], 1e-12, op=Alu.is_lt)
                        V.tensor_scalar(tmpb[:], tmpa[:], -1.0, 1.0, op0=Alu.mult, op1=Alu.add)
                        V.tensor_tensor(den[:], den[:], tmpb[:], op=Alu.mult)
                        V.tensor_single_scalar(tmpa[:], tmpa[:], 1e-12, op=Alu.mult)
                        V.tensor_tensor(den[:], den[:], tmpa[:], op=Alu.add)
                        # u = (dx - Jdg)/den * act (overwrites jdg)
                        for c in range(3):
                            V.tensor_tensor(jdg[c][:], dxt[c][:], jdg[c][:], op=Alu.subtract)
                            V.tensor_tensor(jdg[c][:], jdg[c][:], den[:], op=Alu.divide)
                            V.tensor_tensor(jdg[c][:], jdg[c][:], act[:], op=Alu.mult)
                        # vT into tmpa/tmpb/cmb0 ; Ji += u (x) vT (scratch accv)
                        vt_t = (tmpa, tmpb, cmb0)
                        for c in range(3):
                            V.tensor_tensor(accv[:], dxt[0][:], Ji[0 + c][:], op=Alu.mult)
                            V.tensor_tensor(vt_t[c][:], dxt[1][:], Ji[3 + c][:], op=Alu.mult)
                            V.tensor_tensor(accv[:], accv[:], vt_t[c][:], op=Alu.add)
                            V.tensor_tensor(vt_t[c][:], dxt[2][:], Ji[6 + c][:], op=Alu.mult)
                            V.tensor_tensor(vt_t[c][:], accv[:], vt_t[c][:], op=Alu.add)
                        for r in range(3):
                            for c in range(3):
                                V.tensor_tensor(accv[:], jdg[r][:], vt_t[c][:], op=Alu.mult)
                                V.tensor_tensor(Ji[r * 3 + c][:], Ji[r * 3 + c][:], accv[:], op=Alu.add)
                        # gnn2 = |gx + dgx|^2 ; flags ; gx += act*dgx
                        V.tensor_tensor(tmpa[:], gx[0][:], dgx[0][:], op=Alu.add)
                        V.tensor_tensor(gnn[:], tmpa[:], tmpa[:], op=Alu.mult)
                        for c in (1, 2):
                            V.tensor_tensor(tmpa[:], gx[c][:], dgx[c][:], op=Alu.add)
                            V.tensor_tensor(tmpa[:], tmpa[:], tmpa[:], op=Alu.mult)
                            V.tensor_tensor(gnn[:], gnn[:], tmpa[:], op=Alu.add)
                        V.tensor_single_scalar(tmpa[:], gnn[:], CVG2, op=Alu.is_lt)
                        V.tensor_tensor(tmpa[:], tmpa[:], act[:], op=Alu.mult)
                        V.tensor_tensor(cv[:], cv[:], tmpa[:], op=Alu.max)
                        V.tensor_single_scalar(tmpa[:], gnn[:], DVG2, op=Alu.is_gt)
                        V.tensor_tensor(tmpa[:], tmpa[:], act[:], op=Alu.mult)
                        V.tensor_tensor(dv[:], dv[:], tmpa[:], op=Alu.max)
                        for c in range(3):
                            V.tensor_tensor(tmpa[:], dgx[c][:], act[:], op=Alu.mult)
                            V.tensor_tensor(gx[c][:], gx[c][:], tmpa[:], op=Alu.add)

                    # ---- final ----
                    inm = act
                    for c in range(3):
                        nc.vector.scalar_tensor_tensor(
                            tmpa[:], xc[c][:], offb[:, c:c + 1], rs_bc(c, F),
                            op0=Alu.subtract, op1=Alu.mult)
                        V.tensor_single_scalar(tmpa[:], tmpa[:], 0.0, op=Alu.abs_max)
                        V.tensor_single_scalar(tmpa[:], tmpa[:], 1.0, op=Alu.is_le)
                        if c == 0:
                            V.tensor_copy(inm[:], tmpa[:])
                        else:
                            V.tensor_tensor(inm[:], inm[:], tmpa[:], op=Alu.mult)
                    V.tensor_scalar(tmpa[:], dv[:], -1.0, 1.0, op0=Alu.mult, op1=Alu.add)
                    V.tensor_tensor(inm[:], inm[:], tmpa[:], op=Alu.mult)
                    V.tensor_tensor(inm[:], inm[:], cv[:], op=Alu.mult)
                    dup = dgx[0]
                    d2 = dgx[1]
                    d2b = dgx[2]
                    V.memset(dup[:], 0.0)
                    for i in range(NI - 1):
                        njr = NI - 1 - i

                        def at(t_, off):
                            return mk_ap(t_[:], off, [[pstr(t_), 128], [NI, Tp], [1, njr]])

                        def bc_i(t_):
                            return mk_ap(t_[:], i, [[pstr(t_), 128], [NI, Tp], [0, njr]])
                        sl_d2 = at(d2, 0)
                        sl_d2b = at(d2b, 0)
                        for c in range(3):
                            V.tensor_tensor(sl_d2b, at(xc[c], i + 1), bc_i(xc[c]), op=Alu.subtract)
                            V.tensor_tensor(sl_d2b, sl_d2b, sl_d2b, op=Alu.mult)
                            if c == 0:
                                V.tensor_copy(sl_d2, sl_d2b)
                            else:
                                V.tensor_tensor(sl_d2, sl_d2, sl_d2b, op=Alu.add)
                        V.tensor_single_scalar(sl_d2, sl_d2, DUP_EPS2, op=Alu.is_lt)
                        V.tensor_tensor(sl_d2, sl_d2, bc_i(inm), op=Alu.mult)
                        V.tensor_tensor(at(dup, i + 1), at(dup, i + 1), sl_d2, op=Alu.max)
                    V.tensor_scalar(dup[:], dup[:], -1.0, 1.0, op0=Alu.mult, op1=Alu.add)
                    V.tensor_tensor(inm[:], inm[:], dup[:], op=Alu.mult)
                    vau = wpool.tile([128, F], u8, tag="vau", name="vau")
                    V.tensor_copy(vau[:], inm[:])
                    nc.sync.dma_start(
                        mk_ap(va_out, t0 * NI, [[PPP * NI, 128], [1, F]]), vau[:])
                    aos = gpool.tile([128, F * 9], f32, tag="aos", name="aos")
                    for c in range(3):
                        V.tensor_copy(
                            mk_ap(aos[:], c, [[pstr(aos), 128], [3, F]]), xc[c][:])
                    nc.sync.dma_start(
                        mk_ap(xc_out, t0 * NI * 3, [[PPP * NI * 3, 128], [1, F * 3]]),
                        mk_ap(aos[:], 0, [[pstr(aos), 128], [1, F * 3]]))
                    for k in range(9):
                        V.tensor_copy(
                            mk_ap(aos[:], k, [[pstr(aos), 128], [9, F]]), Ji[k][:])
                    nc.sync.dma_start(
                        mk_ap(ji_out, t0 * NI * 9, [[PPP * NI * 9, 128], [1, F * 9]]),
                        mk_ap(aos[:], 0, [[pstr(aos), 128], [1, F * 9]]))

    nc.finalize()
    return nc


# ---------------- host side ----------------
INIT_BONES = np.array([0, 1, 2, 4, 5, 10, 11, 12, 15, 16, 17, 18, 19], dtype=np.int32)


def make_in_maps(xd, tfs, lbs_voxel, offset, scale, n_cores=8, PPP=196):
    xd = np.asarray(xd, np.float32)
    tfs = np.asarray(tfs, np.float32)
    N = xd.shape[1]
    per = N // n_cores
    N_pad = 128 * PPP
    A = np.ascontiguousarray(tfs[0, :, :3, :4].reshape(NJ, 12)).astype(np.float32)
    tinv = np.linalg.inv(tfs[0][INIT_BONES].astype(np.float32)).astype(np.float32)
    tinv12 = np.ascontiguousarray(tinv[:, :3, :4].reshape(1, NI * 12))
    lbs = np.ascontiguousarray(np.asarray(lbs_voxel, np.float32).reshape(NJ, NCELL))
    off = np.ascontiguousarray(np.asarray(offset, np.float32).reshape(1, 3))
    sc = np.ascontiguousarray(np.asarray(scale, np.float32).reshape(1, 1))
    maps = []
    for c in range(n_cores):
        xs = xd[0, c * per:(c + 1) * per]
        pad = np.zeros((N_pad, 3), np.float32)
        pad[:per] = xs
        maps.append({
            "xd_in": np.ascontiguousarray(pad.reshape(128, PPP * 3)),
            "lbs": lbs, "a_in": A, "tinv_in": tinv12,
            "off_in": off, "scale_in": sc,
        })
    return maps


def assemble(results, N, n_cores=8, PPP=196):
    per = N // n_cores
    xcs, jis, vas = [], [], []
    for c in range(n_cores):
        r = results[c]
        xcs.append(r["xc_out"].reshape(128 * PPP, NI, 3)[:per])
        jis.append(r["ji_out"].reshape(128 * PPP, NI, 9)[:per])
        vas.append(r["va_out"].reshape(128 * PPP, NI)[:per])
    xc = np.concatenate(xcs)[None]
    ji = np.concatenate(jis)[None].reshape(1, N, NI, 3, 3)
    va = np.concatenate(vas)[None].astype(bool)
    return xc, va, ji


# ---------------- harness entry point ----------------
_CACHE = {}


def kernel(xd, tfs, lbs_voxel, offset, scale):
    """Full-input entry: shards points over 8 NeuronCores, returns (xc, valid, Jinv)."""
    from concourse.bass_utils import run_bass_kernel_spmd
    xd = np.asarray(xd, np.float32)
    if "nc" not in _CACHE:
        _CACHE["nc"] = build_nc()
    nc = _CACHE["nc"]
    maps = make_in_maps(xd, np.asarray(tfs, np.float32), np.asarray(lbs_voxel, np.float32),
                        np.asarray(offset, np.float32), np.asarray(scale, np.float32))
    res = run_bass_kernel_spmd(nc, maps, list(range(8))).results
    return assemble(res, xd.shape[1])
